# revision 14
# baseline (speedup 1.0000x reference)
"""ClusterInversionLoss Trainium2 kernel.

Strategy (data-parallel over the flat pair list, per sharding hint):
  - Host: gather each pair's rows, orient every pair so sign=+1 (swap
    i/j when y_i<y_j; ties contribute 0 via wd=0), l0-shift the logits
    (softmax shift invariance), fold |dy|*w_pair into a single wd plane,
    and pack per core a (128, 16384) bf16 matrix whose partition dim
    interleaves 31 pair-groups x 4 shifted logits (+ a constant
    zero-logit row that exp turns into the softmax "+1"), with the i/j
    sides of a pair in adjacent columns.  total_weight is a pure
    function of the inputs (no softmax), summed on host.
  - Device (per core): exp on ACT; Z=1+sum(e) and W=sum(c*e) via
    128x32-column-tiled matmuls on the otherwise-idle Tensor engine;
    1/Z via the single-instruction DVE reciprocal_approx_fast;
    s=W*(1/Z) and delta=s_i-s_j on DVE; softplus(-delta)=ln(1+exp(-d))
    on ACT (exp+ln share one table set); fused multiply-by-wd +
    per-partition reduce on DVE, chained across rounds via the reduce
    initial-value operand.
  - Host: sum the 8x128 loss partials, divide by host total_weight.

Computes exactly the reference quantity; only rows referenced by pairs
contribute, so unpaired rows need not be touched.
"""

import numpy as np

import concourse.bacc as bacc
import concourse.mybir as mybir
from concourse.bass_utils import run_bass_kernel_spmd
from concourse.tile import TileContext

NCORES = 8
NPAIRS = 2_000_000
PC = NPAIRS // NCORES   # 250_000 pairs per core
P = 128

G = 31                  # pair-groups per column (partition = 4*g + c)
ONES_ROW = 124          # constant zero-logit row -> exp() == 1 (the +1 in Z)
F = 16_384              # x columns per core
PC_PAD = (F // 2) * G   # 253_952 padded pairs per core >= PC
SRC = 4_096             # x columns per (full) super-round
NJ = 2                  # PSUM sub-chunks per full super-round (2048 cols)
NK = 4                  # matmul partition-blocks per sub-chunk
MB = 512                # matmul moving free dim (one PSUM bank)
TD = MB // 2            # delta columns per (j, k) block
# Short rounds at the ends shrink pipeline fill (first exp waits on a
# 0.5MB DMA, not 1MB) and the serial drain through the 8-stage tail.
SR_COLS = [2048, 2048, 4096, 4096, 2048, 2048]
NSR = len(SR_COLS)
assert sum(SR_COLS) == F

EPS = 1e-8

f32 = mybir.dt.float32
bf16 = mybir.dt.bfloat16
fp8 = mybir.dt.float8e4
AF = mybir.ActivationFunctionType
ALU = mybir.AluOpType


def _pin_act_tables(arch):
    """Make every ACT function we use first-match to one table set that
    contains both exp and ln, so the kernel needs a single
    ACT_TABLE_LOAD instead of thrashing between the exp-only and
    ln-only sets (1.3us per reload).  Only membership of the cached
    selection dict is edited; set indices (act_func_set_id) and the
    real on-device tables are untouched, so lowering stays correct.
    """
    from concourse.hw_specs import get_activation_tables

    tabs = get_activation_tables(arch)
    ours = {AF.Exp, AF.Ln}
    combined = None
    for name, fns in tabs.items():
        if ours <= fns:
            combined = name
            break
    if combined is None:
        return
    for name, fns in tabs.items():
        if name != combined:
            fns -= ours


def _build():
    nc = bacc.Bacc("TRN2", target_bir_lowering=False)
    _pin_act_tables(nc.m.arch)
    X = nc.dram_tensor("x", [P, F], fp8, kind="ExternalInput")
    WD = nc.dram_tensor("wd", [P, F // 8], bf16, kind="ExternalInput")
    WZT = nc.dram_tensor("wzt", [P, 32], bf16, kind="ExternalInput")
    WWT = nc.dram_tensor("wwt", [P, 32], bf16, kind="ExternalInput")
    OUT = nc.dram_tensor("out", [P, 1], f32, kind="ExternalOutput")

    with TileContext(nc) as tc:
        with (
            tc.tile_pool(name="io", bufs=1) as io,
            tc.tile_pool(name="ew", bufs=1) as ew,
            tc.tile_pool(name="ps", bufs=2, space="PSUM") as ps,
            tc.tile_pool(name="s1", bufs=2) as s1,
            tc.tile_pool(name="cst", bufs=1) as cst,
            tc.tile_pool(name="acc", bufs=1) as accp,
        ):
            sr_off = np.cumsum([0] + SR_COLS[:-1]).tolist()

            # Input DMAs first: the first exp waits on x0, so x wins the
            # queue; wz/ww are tiny; wd (512KB, first read by the sr0
            # reduce) goes after the first two x rounds.
            xts = []
            wdts = []
            for sr in range(NSR):
                cols = SR_COLS[sr]
                xt = io.tile([P, cols], fp8, tag=f"x{sr}", name=f"x{sr}")
                nc.sync.dma_start(out=xt[:],
                                  in_=X[:, sr_off[sr]:sr_off[sr] + cols])
                xts.append(xt)
                if sr == 1:
                    wz = cst.tile([P, 32], bf16, tag="wz", name="wz")
                    nc.sync.dma_start(out=wz[:], in_=WZT[:, :])
                    ww = cst.tile([P, 32], bf16, tag="ww", name="ww")
                    nc.sync.dma_start(out=ww[:], in_=WWT[:, :])
                wt = cst.tile([P, cols // 8], bf16, tag=f"wd{sr}",
                              name=f"wd{sr}")
                wcol = sr_off[sr] // 8
                nc.sync.dma_start(out=wt[:],
                                  in_=WD[:, wcol:wcol + cols // 8])
                wdts.append(wt)

            accs = [accp.tile([P, 1], f32, tag=f"acc{i}", name=f"acc{i}")
                    for i in range(NSR)]

            def super_round(sr):
                cols = SR_COLS[sr]
                nj = cols // (NK * MB)
                xt = xts[sr]
                E = ew.tile([P, cols], bf16, tag=f"E{sr}", name=f"E{sr}")
                nc.scalar.activation(E[:], xt[:], AF.Exp)

                Zt = ps.tile([P, NJ, MB], f32, tag="Z", name=f"Z{sr}")
                Wt = ps.tile([P, NJ, MB], f32, tag="W", name=f"W{sr}")
                for j in range(nj):
                    for k in range(NK):
                        rhs = E[:, j * (NK * MB) + k * MB:
                                j * (NK * MB) + (k + 1) * MB]
                        nc.tensor.matmul(
                            Zt[32 * k:32 * (k + 1), j], wz[:, :], rhs,
                            start=True, stop=True, tile_position=(0, 32 * k))
                    for k in range(NK):
                        rhs = E[:, j * (NK * MB) + k * MB:
                                j * (NK * MB) + (k + 1) * MB]
                        nc.tensor.matmul(
                            Wt[32 * k:32 * (k + 1), j], ww[:, :], rhs,
                            start=True, stop=True, tile_position=(0, 32 * k))

                RZ = s1.tile([P, NJ, MB], f32, tag="RZ", name=f"RZ{sr}")
                nc.vector.reciprocal_approx_fast(out=RZ[:, :nj],
                                                 in_=Zt[:, :nj])
                S = s1.tile([P, NJ, TD, 2], bf16, tag="S", name=f"S{sr}")
                nc.vector.tensor_mul(out=S[:, :nj], in0=Wt[:, :nj],
                                     in1=RZ[:, :nj])
                D = s1.tile([P, NJ, TD], bf16, tag="D", name=f"D{sr}")
                nc.vector.tensor_sub(out=D[:, :nj], in0=S[:, :nj, :, 0],
                                     in1=S[:, :nj, :, 1])
                U = s1.tile([P, NJ, TD], bf16, tag="U", name=f"U{sr}")
                nc.scalar.activation(U[:, :nj], D[:, :nj], AF.Exp, scale=-1.0)
                SP = s1.tile([P, NJ, TD], bf16, tag="SP", name=f"SP{sr}")
                nc.scalar.activation(SP[:, :nj], U[:, :nj], AF.Ln, bias=1.0)
                SC = s1.tile([P, NJ, TD], bf16, tag="SC", name=f"SC{sr}")
                nc.vector.scalar_tensor_tensor(
                    out=SC[:, :nj], in0=SP[:, :nj], scalar=1.0,
                    in1=wdts[sr][:], op0=ALU.mult, op1=ALU.mult,
                    accum_out=accs[sr][:])
                if sr > 0:
                    nc.vector.tensor_add(out=accs[sr][:], in0=accs[sr][:],
                                         in1=accs[sr - 1][:])

            for sr in range(NSR):
                super_round(sr)

            nc.sync.dma_start(out=OUT[:, :], in_=accs[NSR - 1][:])

    nc.compile()
    return nc


_NC_CACHE = {}


def _get_nc():
    if "nc" not in _NC_CACHE:
        _NC_CACHE["nc"] = _build()
    return _NC_CACHE["nc"]


def _weights():
    # lhsT [K=128, M=32]: column g (< G) sums the 4 class-exps of group g;
    # WZT also picks up the constant-1 row (softmax +1).  Column 31 is a
    # padding output fed by all rows so its Z/W stay wholesome (no 1/0 in
    # reciprocal); its wd is always 0 so it never contributes.
    wzt = np.zeros((P, 32), np.float32)
    wwt = np.zeros((P, 32), np.float32)
    for g in range(G):
        for c in range(4):
            wzt[4 * g + c, g] = 1.0
            wwt[4 * g + c, g] = float(c + 1)
    wzt[ONES_ROW, :G] = 1.0
    wzt[:, 31] = 1.0
    wwt[:, 31] = 1.0
    return wzt, wwt


def _prepare(inputs, targets, cluster_ids, sample_weight, pair_i, pair_j):
    import ml_dtypes

    bf = ml_dtypes.bfloat16
    x = np.ascontiguousarray(np.asarray(inputs), dtype=np.float32)
    t = np.asarray(targets)
    w = np.asarray(sample_weight, dtype=np.float32)
    pi = np.asarray(pair_i).astype(np.int64, copy=False)
    pj = np.asarray(pair_j).astype(np.int64, copy=False)

    dy = (t[pi] - t[pj]).astype(np.int64)
    swap = dy < 0
    pi2 = np.where(swap, pj, pi)
    pj2 = np.where(swap, pi, pj)
    dist = np.abs(dy).astype(np.float32)

    li = x[pi2]                         # (NPAIRS, 5), oriented so s_i-s_j
    lj = x[pj2]
    lsi = li[:, 1:5] - li[:, 0:1]       # l0-shift: softmax shift-invariant
    lsj = lj[:, 1:5] - lj[:, 0:1]

    wp = 0.5 * (w[pi] + w[pj])          # symmetric under swap
    wd = dist * wp                      # 0 exactly for ties (inactive)
    twa = float((wp * (dist != 0)).sum(dtype=np.float64))

    wzt, wwt = _weights()
    wzt = wzt.astype(bf)
    wwt = wwt.astype(bf)

    B = F // 2
    maps = []
    for kcore in range(NCORES):
        sl = slice(kcore * PC, (kcore + 1) * PC)

        lsi_p = np.zeros((PC_PAD, 4), np.float32)
        lsi_p[:PC] = lsi[sl]
        lsj_p = np.zeros((PC_PAD, 4), np.float32)
        lsj_p[:PC] = lsj[sl]
        wd_p = np.zeros(PC_PAD, np.float32)
        wd_p[:PC] = wd[sl]

        # x_dev[4g+c, 2b+side] = logit c of side of pair q = G*b+g
        lsi_r = lsi_p.reshape(B, G, 4)          # [b, g, c]
        lsj_r = lsj_p.reshape(B, G, 4)
        x4 = np.stack([lsi_r, lsj_r], axis=3)   # [b, g, c, side]
        x_dev = np.zeros((P, F), np.float32)
        x_dev[:4 * G] = x4.transpose(1, 2, 0, 3).reshape(4 * G, F)
        x_dev = np.ascontiguousarray(x_dev).astype(ml_dtypes.float8_e4m3)

        # wd_dev[32k+g, u*TD+t] = wd[q], q = G*(u*4*TD + k*TD + t) + g,
        # where u indexes the 8 uniform 2048-column sub-chunks.
        NU = F // 2048
        wd_r = wd_p.reshape(NU, NK, TD, G)       # [u, k, t, g]
        wd_r = wd_r.transpose(1, 3, 0, 2)        # [k, g, u, t]
        wd_dev = np.zeros((NK, 32, NU, TD), np.float32)
        wd_dev[:, :G] = wd_r
        wd_dev = np.ascontiguousarray(
            wd_dev.reshape(P, F // 8)).astype(bf)

        maps.append({"x": x_dev, "wd": wd_dev, "wzt": wzt, "wwt": wwt})
    return maps, twa


def _run(in_maps, trace=False, **kw):
    nc = _get_nc()
    return run_bass_kernel_spmd(nc, in_maps, list(range(NCORES)), trace=trace, **kw)


def kernel(inputs, targets, cluster_ids, sample_weight, pair_i, pair_j):
    in_maps, twa = _prepare(inputs, targets, cluster_ids, sample_weight,
                            pair_i, pair_j)
    res = _run(in_maps)
    tl = 0.0
    for k in range(NCORES):
        o = res.results[k]["out"]
        tl += float(o[:, 0].sum(dtype=np.float64))
    return np.float32(tl / (twa + EPS))


# revision 20
# speedup vs baseline: 1.1942x; 1.1942x over previous
"""ClusterInversionLoss Trainium2 kernel.

Strategy (data-parallel over the flat pair list, per sharding hint):
  - Host: gather each pair's rows, orient every pair so sign=+1 (swap
    i/j when y_i<y_j; ties contribute 0 via wd=0), l0-shift the logits
    (softmax shift invariance), fold |dy|*w_pair into a single wd plane,
    and pack per core a (128, 16384) bf16 matrix whose partition dim
    interleaves 31 pair-groups x 4 shifted logits (+ a constant
    zero-logit row that exp turns into the softmax "+1"), with the i/j
    sides of a pair in adjacent columns.  total_weight is a pure
    function of the inputs (no softmax), summed on host.
  - Device (per core): exp on ACT; Z=1+sum(e) and W=sum(c*e) via
    128x32-column-tiled matmuls on the otherwise-idle Tensor engine;
    1/Z via the single-instruction DVE reciprocal_approx_fast;
    s=W*(1/Z) and delta=s_i-s_j on DVE; softplus(-delta)=ln(1+exp(-d))
    on ACT (exp+ln share one table set); fused multiply-by-wd +
    per-partition reduce on DVE, chained across rounds via the reduce
    initial-value operand.
  - Host: sum the 8x128 loss partials, divide by host total_weight.

Computes exactly the reference quantity; only rows referenced by pairs
contribute, so unpaired rows need not be touched.
"""

import numpy as np

import concourse.bacc as bacc
import concourse.mybir as mybir
from concourse.bass_utils import run_bass_kernel_spmd
from concourse.tile import TileContext

NCORES = 8
NPAIRS = 2_000_000
PC = NPAIRS // NCORES   # 250_000 pairs per core
P = 128

G = 31                  # pair-groups per column (partition = 4*g + c)
ONES_ROW = 124          # constant zero-logit row -> exp() == 1 (the +1 in Z)
# Only ACTIVE pairs (dist != 0) are shipped to the device -- inactive
# pairs contribute exactly 0 to both sums.  ~79% of the 2M pairs are
# active (~198k/core after even split); capacity below is 222k/core.
F = 14_336              # x columns per core
PC_PAD = (F // 2) * G   # 222_208 padded pair slots per core
NJ = 2                  # PSUM sub-chunks per full super-round (2048 cols)
NK = 4                  # matmul partition-blocks per sub-chunk
MB = 512                # matmul moving free dim (one PSUM bank)
TD = MB // 2            # delta columns per (j, k) block
# Short rounds at the ends shrink pipeline fill (first exp waits on a
# 0.5MB DMA, not 1MB) and the serial drain through the 8-stage tail.
SR_COLS = [2048, 2048, 4096, 4096, 2048]
NSR = len(SR_COLS)
assert sum(SR_COLS) == F

EPS = 1e-8

f32 = mybir.dt.float32
bf16 = mybir.dt.bfloat16
fp8 = mybir.dt.float8e4
AF = mybir.ActivationFunctionType
ALU = mybir.AluOpType


def _pin_act_tables(arch):
    """Make every ACT function we use first-match to one table set that
    contains both exp and ln, so the kernel needs a single
    ACT_TABLE_LOAD instead of thrashing between the exp-only and
    ln-only sets (1.3us per reload).  Only membership of the cached
    selection dict is edited; set indices (act_func_set_id) and the
    real on-device tables are untouched, so lowering stays correct.
    """
    from concourse.hw_specs import get_activation_tables

    tabs = get_activation_tables(arch)
    ours = {AF.Exp, AF.Ln}
    combined = None
    for name, fns in tabs.items():
        if ours <= fns:
            combined = name
            break
    if combined is None:
        return
    for name, fns in tabs.items():
        if name != combined:
            fns -= ours


def _build():
    nc = bacc.Bacc("TRN2", target_bir_lowering=False)
    _pin_act_tables(nc.m.arch)
    X = nc.dram_tensor("x", [P, F], bf16, kind="ExternalInput")
    WD = nc.dram_tensor("wd", [P, F // 8], bf16, kind="ExternalInput")
    WZT = nc.dram_tensor("wzt", [P, 32], bf16, kind="ExternalInput")
    WWT = nc.dram_tensor("wwt", [P, 32], bf16, kind="ExternalInput")
    OUT = nc.dram_tensor("out", [P, 1], f32, kind="ExternalOutput")

    with TileContext(nc) as tc:
        with (
            tc.tile_pool(name="io", bufs=1) as io,
            tc.tile_pool(name="ew", bufs=1) as ew,
            tc.tile_pool(name="ps", bufs=2, space="PSUM") as ps,
            tc.tile_pool(name="s1", bufs=2) as s1,
            tc.tile_pool(name="cst", bufs=1) as cst,
            tc.tile_pool(name="acc", bufs=1) as accp,
        ):
            sr_off = np.cumsum([0] + SR_COLS[:-1]).tolist()

            # Input DMAs first: the first exp waits on x0, so x wins the
            # queue; wz/ww are tiny; wd (512KB, first read by the sr0
            # reduce) goes after the first two x rounds.
            # DMA issue order: x wins the front of the queue (the exps
            # gate everything); wd slices are interleaved just-in-time.
            xts = []
            wdts = []

            def emit_x(sr):
                cols = SR_COLS[sr]
                xt = io.tile([P, cols], bf16, tag=f"x{sr}", name=f"x{sr}")
                nc.sync.dma_start(out=xt[:],
                                  in_=X[:, sr_off[sr]:sr_off[sr] + cols])
                xts.append(xt)

            def emit_wd(sr):
                cols = SR_COLS[sr]
                wt = cst.tile([P, cols // 8], bf16, tag=f"wd{sr}",
                              name=f"wd{sr}")
                wcol = sr_off[sr] // 8
                nc.sync.dma_start(out=wt[:],
                                  in_=WD[:, wcol:wcol + cols // 8])
                wdts.append(wt)

            for sr in range(3):
                emit_x(sr)
            wz = cst.tile([P, 32], bf16, tag="wz", name="wz")
            nc.sync.dma_start(out=wz[:], in_=WZT[:, :])
            ww = cst.tile([P, 32], bf16, tag="ww", name="ww")
            nc.sync.dma_start(out=ww[:], in_=WWT[:, :])
            emit_wd(0)
            emit_wd(1)
            emit_x(3)
            emit_wd(2)
            emit_x(4)
            emit_wd(3)
            emit_wd(4)

            accs = [accp.tile([P, 1], f32, tag=f"acc{i}", name=f"acc{i}")
                    for i in range(NSR)]

            def super_round(sr):
                cols = SR_COLS[sr]
                nj = cols // (NK * MB)
                xt = xts[sr]
                E = ew.tile([P, cols], bf16, tag=f"E{sr}", name=f"E{sr}")
                nc.scalar.activation(E[:], xt[:], AF.Exp)

                Zt = ps.tile([P, NJ, MB], f32, tag="Z", name=f"Z{sr}")
                Wt = ps.tile([P, NJ, MB], f32, tag="W", name=f"W{sr}")
                for j in range(nj):
                    for k in range(NK):
                        rhs = E[:, j * (NK * MB) + k * MB:
                                j * (NK * MB) + (k + 1) * MB]
                        nc.tensor.matmul(
                            Zt[32 * k:32 * (k + 1), j], wz[:, :], rhs,
                            start=True, stop=True, tile_position=(0, 32 * k))
                    for k in range(NK):
                        rhs = E[:, j * (NK * MB) + k * MB:
                                j * (NK * MB) + (k + 1) * MB]
                        nc.tensor.matmul(
                            Wt[32 * k:32 * (k + 1), j], ww[:, :], rhs,
                            start=True, stop=True, tile_position=(0, 32 * k))

                RZ = s1.tile([P, NJ, MB], f32, tag="RZ", name=f"RZ{sr}")
                nc.vector.reciprocal_approx_fast(out=RZ[:, :nj],
                                                 in_=Zt[:, :nj])
                S = s1.tile([P, NJ, TD, 2], bf16, tag="S", name=f"S{sr}")
                nc.vector.tensor_mul(out=S[:, :nj], in0=Wt[:, :nj],
                                     in1=RZ[:, :nj])
                D = s1.tile([P, NJ, TD], bf16, tag="D", name=f"D{sr}")
                nc.vector.tensor_sub(out=D[:, :nj], in0=S[:, :nj, :, 0],
                                     in1=S[:, :nj, :, 1])
                U = s1.tile([P, NJ, TD], bf16, tag="U", name=f"U{sr}")
                nc.scalar.activation(U[:, :nj], D[:, :nj], AF.Exp, scale=-1.0)
                SP = s1.tile([P, NJ, TD], bf16, tag="SP", name=f"SP{sr}")
                nc.scalar.activation(SP[:, :nj], U[:, :nj], AF.Ln, bias=1.0)
                SC = s1.tile([P, NJ, TD], bf16, tag="SC", name=f"SC{sr}")
                nc.vector.scalar_tensor_tensor(
                    out=SC[:, :nj], in0=SP[:, :nj], scalar=1.0,
                    in1=wdts[sr][:], op0=ALU.mult, op1=ALU.mult,
                    accum_out=accs[sr][:])
                if sr > 0:
                    nc.vector.tensor_add(out=accs[sr][:], in0=accs[sr][:],
                                         in1=accs[sr - 1][:])

            for sr in range(NSR):
                super_round(sr)

            nc.sync.dma_start(out=OUT[:, :], in_=accs[NSR - 1][:])

    nc.compile()
    return nc


_NC_CACHE = {}


def _get_nc():
    if "nc" not in _NC_CACHE:
        _NC_CACHE["nc"] = _build()
    return _NC_CACHE["nc"]


def _weights():
    # lhsT [K=128, M=32]: column g (< G) sums the 4 class-exps of group g;
    # WZT also picks up the constant-1 row (softmax +1).  Column 31 is a
    # padding output fed by all rows so its Z/W stay wholesome (no 1/0 in
    # reciprocal); its wd is always 0 so it never contributes.
    wzt = np.zeros((P, 32), np.float32)
    wwt = np.zeros((P, 32), np.float32)
    for g in range(G):
        for c in range(4):
            wzt[4 * g + c, g] = 1.0
            wwt[4 * g + c, g] = float(c + 1)
    wzt[ONES_ROW, :G] = 1.0
    wzt[:, 31] = 1.0
    wwt[:, 31] = 1.0
    return wzt, wwt


def _prepare(inputs, targets, cluster_ids, sample_weight, pair_i, pair_j):
    import ml_dtypes

    bf = ml_dtypes.bfloat16
    x = np.ascontiguousarray(np.asarray(inputs), dtype=np.float32)
    t = np.asarray(targets)
    w = np.asarray(sample_weight, dtype=np.float32)
    pi = np.asarray(pair_i).astype(np.int64, copy=False)
    pj = np.asarray(pair_j).astype(np.int64, copy=False)

    dy = (t[pi] - t[pj]).astype(np.int64)
    wp = 0.5 * (w[pi] + w[pj])          # symmetric under swap
    act = dy != 0
    twa = float((wp * act).sum(dtype=np.float64))

    # keep only active pairs, oriented so sign=+1 (s_i - s_j)
    idx = np.flatnonzero(act)
    dyA = dy[idx]
    swap = dyA < 0
    piA = np.where(swap, pj[idx], pi[idx])
    pjA = np.where(swap, pi[idx], pj[idx])
    dist = np.abs(dyA).astype(np.float32)

    li = x[piA]
    lj = x[pjA]
    lsi = li[:, 1:5] - li[:, 0:1]       # l0-shift: softmax shift-invariant
    lsj = lj[:, 1:5] - lj[:, 0:1]
    wd = dist * wp[idx]

    nact = len(idx)
    assert nact <= NCORES * PC_PAD, f"active pairs {nact} exceed capacity"
    cpc = (nact + NCORES - 1) // NCORES  # active pairs per core (even split)

    wzt, wwt = _weights()
    wzt = wzt.astype(bf)
    wwt = wwt.astype(bf)

    B = F // 2
    maps = []
    for kcore in range(NCORES):
        lo = kcore * cpc
        hi = min(lo + cpc, nact)
        n = hi - lo

        lsi_p = np.zeros((PC_PAD, 4), np.float32)
        lsi_p[:n] = lsi[lo:hi]
        lsj_p = np.zeros((PC_PAD, 4), np.float32)
        lsj_p[:n] = lsj[lo:hi]
        wd_p = np.zeros(PC_PAD, np.float32)
        wd_p[:n] = wd[lo:hi]

        # x_dev[4g+c, 2b+side] = logit c of side of pair q = G*b+g
        lsi_r = lsi_p.reshape(B, G, 4)          # [b, g, c]
        lsj_r = lsj_p.reshape(B, G, 4)
        x4 = np.stack([lsi_r, lsj_r], axis=3)   # [b, g, c, side]
        x_dev = np.zeros((P, F), np.float32)
        x_dev[:4 * G] = x4.transpose(1, 2, 0, 3).reshape(4 * G, F)
        x_dev = np.ascontiguousarray(x_dev).astype(bf)

        # wd_dev[32k+g, u*TD+t] = wd[q], q = G*(u*4*TD + k*TD + t) + g,
        # where u indexes the 8 uniform 2048-column sub-chunks.
        NU = F // 2048
        wd_r = wd_p.reshape(NU, NK, TD, G)       # [u, k, t, g]
        wd_r = wd_r.transpose(1, 3, 0, 2)        # [k, g, u, t]
        wd_dev = np.zeros((NK, 32, NU, TD), np.float32)
        wd_dev[:, :G] = wd_r
        wd_dev = np.ascontiguousarray(
            wd_dev.reshape(P, F // 8)).astype(bf)

        maps.append({"x": x_dev, "wd": wd_dev, "wzt": wzt, "wwt": wwt})
    return maps, twa


def _run(in_maps, trace=False, **kw):
    nc = _get_nc()
    return run_bass_kernel_spmd(nc, in_maps, list(range(NCORES)), trace=trace, **kw)


def kernel(inputs, targets, cluster_ids, sample_weight, pair_i, pair_j):
    in_maps, twa = _prepare(inputs, targets, cluster_ids, sample_weight,
                            pair_i, pair_j)
    res = _run(in_maps)
    tl = 0.0
    for k in range(NCORES):
        o = res.results[k]["out"]
        tl += float(o[:, 0].sum(dtype=np.float64))
    return np.float32(tl / (twa + EPS))
